# revision 61
# baseline (speedup 1.0000x reference)
"""Sparse GQA attention (nn_MHA_13950053777893) on 8 TRN2 NeuronCores.

Problem: B=2, Sq=Sk=2048, H=16 q-heads, Hkv=4, D=128, f32.
Reference semantics (prefix-valid key padding mask of length sk per batch):
  - score(t, s) = q.k/sqrt(D) for s <= t + sk - Sq, else exactly -10000
  - softmax over s; rows t < Sq - sk are all -10000 -> uniform attention =
    mean over ALL Sk value rows (host fills those rows in f32).

Sharding (no collectives, disjoint outputs):
  core c in 0..7: kv group g = c // 2, heads {4g + 2*(c%2), 4g + 2*(c%2) + 1}
  for BOTH batches -> 4 head-instances per core, identical work per core.

Device algorithm per head-instance (S^T layout, bf16 matmuls, f32 psum,
software-pipelined with QK two units ahead of AV/den):
  flat list of (chunk, s-block-pair) units over 256-wide t-chunks (chunks
  fully below the causal band are skipped); per unit:
    S^T_psum[128s, 512]  = two QK matmuls (bf16 K-block stationary)
                           (diag block: only the un-masked column suffix)
    P^T bf16[128, 512]   = one exp activation over the pair       (ACT)
    diag block: affine_select(P^T[:, :mhi], 0) masked-prefix only (GPSIMD)
    outT_psum[128d,256] += V_block.T @ P^T seg  (x2, accumulated) (PE)
    den_psum[1,512]     += ones.T @ P^T pair    (one, accumulated)(PE)
  mask-crossing segments' AV/den (and the chunk tail they carry) are
  deferred one unit as FIFO closures flushed after the next unit's QK,
  so the exp->affine chain never stalls the in-order PE; masked pairs
  are a suffix of each chunk, so emission order and psum accumulation
  start/stop flags are preserved.
  chunk end (deferred one chunk, off the PE critical path):
    fold den pair halves (tensor_reduce) + reciprocal_approx_fast (DVE)
    recb = partition_broadcast(rec)                               (GPSIMD)
    posb bf16 = outT_psum * recb  (normalize fused into the cast) (DVE)
    two PE transposes posb -> [t,d] bf16 psum, DVE copy to SBUF,
    one 64KB DMA to out[t0:t0+256, :] (bf16)
Startup: PE warmed with DMA-independent transposes; GPSIMD 'attn' library
load kicked off early (non-blocking); inputs DMA'd in first-use order.
Host: casts inputs to bf16, upcasts the bf16 output to f32, fills the
uniform (t < Sq-sk) rows from the original f32 V.
"""

import functools

import numpy as np

B, SQ, SK, H, HKV, D = 2, 2048, 2048, 16, 4, 128
TC = 256  # t-chunk width
SB = 128  # s-block height
N_CORES = 8


@functools.lru_cache(maxsize=4)
def _build(sk_tuple):
    import concourse.bass as bass  # noqa: F401
    import concourse.mybir as mybir
    from concourse.tile import TileContext
    from concourse import bacc

    F32 = mybir.dt.float32
    BF16 = mybir.dt.bfloat16
    sks = list(sk_tuple)

    nc = bacc.Bacc(target_bir_lowering=False, debug=False)
    qt_d = nc.dram_tensor("qt", [B, 2, D, SQ], BF16, kind="ExternalInput")
    kt_d = nc.dram_tensor("kt", [B, D, SK], BF16, kind="ExternalInput")
    v_d = nc.dram_tensor("v", [B, SK, D], BF16, kind="ExternalInput")
    ident_d = nc.dram_tensor("ident", [128, 128], BF16, kind="ExternalInput")
    out_d = nc.dram_tensor("out", [B, 2, SQ, D], BF16, kind="ExternalOutput")

    scale = float(1.0 / np.sqrt(D))

    with TileContext(nc) as tc:
        with (
            tc.tile_pool(name="big", bufs=1) as big,
            tc.tile_pool(name="pt", bufs=5) as ptp,
            tc.tile_pool(name="posb", bufs=9) as pop,
            tc.tile_pool(name="dsb", bufs=4) as dsp,
            tc.tile_pool(name="eps", bufs=2) as eps,
            tc.tile_pool(name="psS", bufs=3, space="PSUM") as psS,
            tc.tile_pool(name="psO", bufs=2, space="PSUM") as psO,
            tc.tile_pool(name="psD", bufs=1, space="PSUM") as psD,
            tc.tile_pool(name="psT", bufs=2, space="PSUM") as psT,
        ):
            ones_bf = nc.const_aps.aps[(BF16, 1.0)]  # [128,1] bf16 const

            # PE warmup with no DMA dependency: memset a bf16 tile, then
            # stream transposes through the PE while input DMAs land.
            wtile = big.tile([128, 128], BF16, tag="warm")
            nc.gpsimd.memset(wtile, 1.0)
            # kick off the GPSIMD 'attn' library load (partition_broadcast)
            # without a blocking op; affine_select needs no library
            from concourse import library_config
            nc.gpsimd.load_library(library_config.attn)
            pw = psT.tile([128, 256], BF16, tag="ptr", name="pw")
            for _ in range(28):
                nc.tensor.transpose(pw[:, 0:128], wtile, wtile)

            # all input DMAs upfront, in first-use priority order
            kt = {}
            vt = {}
            qts = {}
            ident = big.tile([128, 128], BF16, tag="ident")
            for b in range(B):
                kt[b] = big.tile([D, SK], BF16, tag=f"kt{b}", name=f"kt{b}")
                vt[b] = big.tile([128, SK // 128, D], BF16, tag=f"vt{b}", name=f"vt{b}")
                for hh in range(2):
                    qts[(b, hh)] = big.tile(
                        [D, SQ], BF16, tag=f"qt{b}{hh}", name=f"qt{b}{hh}"
                    )
            q4 = SK // 4
            vre0 = v_d[0].rearrange("(i p) d -> p i d", p=128)
            vre1 = v_d[1].rearrange("(i p) d -> p i d", p=128)
            nc.sync.dma_start(out=kt[0][:, 0:q4], in_=kt_d[0][:, 0:q4])
            nc.sync.dma_start(out=qts[(0, 0)][:, 0:q4], in_=qt_d[0, 0][:, 0:q4])
            nc.sync.dma_start(out=vt[0][:, 0:4, :], in_=vre0[:, 0:4, :])
            nc.sync.dma_start(out=qts[(0, 0)][:, q4 : 2 * q4], in_=qt_d[0, 0][:, q4 : 2 * q4])
            nc.sync.dma_start(out=kt[0][:, q4 : 2 * q4], in_=kt_d[0][:, q4 : 2 * q4])
            nc.sync.dma_start(out=vt[0][:, 4:8, :], in_=vre0[:, 4:8, :])
            nc.sync.dma_start(out=kt[0][:, 2 * q4 :], in_=kt_d[0][:, 2 * q4 :])
            nc.sync.dma_start(out=qts[(0, 0)][:, 2 * q4 :], in_=qt_d[0, 0][:, 2 * q4 :])
            nc.sync.dma_start(out=vt[0][:, 8:, :], in_=vre0[:, 8:, :])
            nc.sync.dma_start(out=ident, in_=ident_d[:, :])
            nc.sync.dma_start(out=qts[(0, 1)][:, :], in_=qt_d[0, 1][:, :])
            nc.sync.dma_start(out=kt[1][:, :], in_=kt_d[1][:, :])
            nc.sync.dma_start(out=qts[(1, 0)][:, :], in_=qt_d[1, 0][:, :])
            nc.sync.dma_start(out=vt[1][:, :, :], in_=vre1[:, :, :])
            nc.sync.dma_start(out=qts[(1, 1)][:, :], in_=qt_d[1, 1][:, :])

            for b in range(B):
                sk = sks[b]
                lo = SQ - sk  # first row with a non-empty band
                for hh in range(2):
                    qt = qts[(b, hh)]
                    # drain the normalize queue eagerly on the final head so
                    # its tail is not a serialized backlog
                    nrm_depth = 2 if (b, hh) == (1, 1) else 3
                    # active chunks and flat (chunk, pair) unit list
                    chunks = []
                    for t0 in range(0, SQ, TC):
                        t_hi = t0 + TC - 1
                        if t_hi < lo:
                            continue
                        w = min(sk, t_hi + sk - SQ + 1)
                        chunks.append((t0, (w + SB - 1) // SB))
                    nch = len(chunks)
                    units = []
                    for c, (t0, nblk) in enumerate(chunks):
                        npair = (nblk + 1) // 2
                        for pi in range(npair):
                            units.append((c, t0, nblk, pi))
                    nu = len(units)

                    ps_tiles = [None] * nu
                    po = pd = None
                    pending_nrm = []
                    pending_diag = []

                    def flush_diag():
                        while pending_diag:
                            fn = pending_diag.pop(0)
                            fn()

                    def flush_nrm():
                        rec0, po0, c0, t00 = pending_nrm.pop(0)
                        recb = dsp.tile([128, TC], F32, tag="recb")
                        nc.gpsimd.partition_broadcast(recb, rec0)
                        posb = pop.tile([128, TC], BF16, tag="posb")
                        nc.vector.tensor_tensor(
                            out=posb, in0=po0[:, 0:256], in1=recb,
                            op=mybir.AluOpType.mult,
                        )
                        ptr = psT.tile([128, 256], BF16, tag="ptr")
                        stn = eps.tile([128, 256], BF16, tag="stn")
                        for j in range(2):
                            if t00 + 128 * j + 127 < lo:
                                continue  # host fills these rows
                            nc.tensor.transpose(
                                ptr[:, 128 * j : 128 * (j + 1)],
                                posb[:, 128 * j : 128 * (j + 1)],
                                ident,
                            )
                            nc.vector.tensor_copy(
                                stn[:, 128 * j : 128 * (j + 1)],
                                ptr[:, 128 * j : 128 * (j + 1)],
                            )
                        odst = out_d[b, hh, t00 : t00 + TC, :].rearrange(
                            "(j p) d -> p j d", p=128
                        )
                        nc.sync.dma_start(
                            out=odst, in_=stn.rearrange("p (j d) -> p j d", j=2)
                        )

                    def emit_qk(u):
                        c, t0, nblk, pi = units[u]
                        i0, i1 = 2 * pi, 2 * pi + 1
                        ps = psS.tile([128, 512], F32, tag="ps")
                        ps_tiles[u] = ps
                        qk_lo = 0
                        if i0 == nblk - 1 and SB * i0 + SB - 1 > t0 + sk - SQ:
                            qk_lo = max(0, min(TC, SB * i0 + (SQ - sk) - t0)) & ~1
                        nc.tensor.matmul(
                            ps[:, qk_lo:256],
                            kt[b][:, SB * i0 : SB * i0 + SB],
                            qt[:, t0 + qk_lo : t0 + TC],
                            start=True, stop=True,
                        )
                        if i1 < nblk:
                            qk_lo1 = 0
                            if i1 == nblk - 1 and SB * i1 + SB - 1 > t0 + sk - SQ:
                                qk_lo1 = max(0, min(TC, SB * i1 + (SQ - sk) - t0)) & ~1
                            nc.tensor.matmul(
                                ps[:, 256 + qk_lo1 : 512],
                                kt[b][:, SB * i1 : SB * i1 + SB],
                                qt[:, t0 + qk_lo1 : t0 + TC],
                                start=True, stop=True,
                            )

                    emit_qk(0)
                    if nu > 1:
                        emit_qk(1)
                    seen_chunks = set()
                    for u in range(nu):
                        c, t0, nblk, pi = units[u]
                        i0, i1 = 2 * pi, 2 * pi + 1
                        have2 = i1 < nblk
                        wseg = 512 if have2 else 256
                        first_of_chunk = c not in seen_chunks
                        seen_chunks.add(c)
                        last_of_chunk = (u + 1 >= nu) or (units[u + 1][0] != c)
                        if u + 2 < nu:
                            emit_qk(u + 2)
                        flush_diag()
                        ps = ps_tiles[u]
                        ps_tiles[u] = None
                        pt = ptp.tile([128, 512], BF16, tag="pt")
                        # cols j < s0 + p + lo - t0 are masked; widest at p=127
                        exp_lo = 0
                        if i0 == nblk - 1 and SB * i0 + SB - 1 > t0 + sk - SQ:
                            # diagonal is segment 0: skip exp on the fully
                            # masked prefix (affine_select fills it with 0)
                            exp_lo = max(0, min(TC, SB * i0 + (SQ - sk) - t0))
                            exp_lo &= ~1
                        if False:
                            # Schraudolph exp on DVE (bf16 bits via int16):
                            # i16 = ps*scale*(2^7/ln2) + (127*2^7 - 5.59)
                            nc.vector.tensor_scalar(
                                out=pt[:, 0:wseg].bitcast(mybir.dt.int16),
                                in0=ps[:, 0:wseg],
                                scalar1=float(scale * 128.0 / np.log(2.0)),
                                scalar2=float(127.0 * 128.0 - 5.59),
                                op0=mybir.AluOpType.mult,
                                op1=mybir.AluOpType.add,
                            )
                        else:
                            nc.scalar.activation(
                                out=pt[:, exp_lo:wseg],
                                in_=ps[:, exp_lo:wseg],
                                func=mybir.ActivationFunctionType.Exp,
                                scale=scale,
                            )
                        for j, i in ((0, i0), (256, i1)):
                            if i >= nblk:
                                continue
                            s0 = SB * i
                            if s0 + SB - 1 > t0 + sk - SQ:
                                # zero entries with (t0+jj)-(s0+p)-(SQ-sk) < 0
                                # only cols [0, mhi) can be masked (p<=127)
                                mhi = min(TC, s0 + 127 + (SQ - sk) - t0)
                                nc.gpsimd.affine_select(
                                    out=pt[:, j : j + mhi],
                                    in_=pt[:, j : j + mhi],
                                    compare_op=mybir.AluOpType.is_ge,
                                    fill=0.0,
                                    base=t0 - s0 - (SQ - sk),
                                    channel_multiplier=-1,
                                    pattern=[[1, mhi]],
                                )
                        if first_of_chunk:
                            po = psO.tile([128, 512], F32, tag="po")
                            pd = psD.tile([1, 512], F32, tag="pd")
                        any_masked = False
                        for j, i in ((0, i0), (256, i1)):
                            if i >= nblk:
                                continue
                            masked = SB * i + SB - 1 > t0 + sk - SQ
                            any_masked = any_masked or masked
                            def em_av(po0=po, vt0=vt[b][:, i, :],
                                      pt0=pt[:, j : j + 256],
                                      st=first_of_chunk and (j == 0),
                                      sp=last_of_chunk and (i == i1 or not have2)):
                                nc.tensor.matmul(po0[:, 0:256], vt0, pt0,
                                                 start=st, stop=sp)
                            if masked:
                                pending_diag.append(em_av)
                            else:
                                em_av()
                        def em_den(pd0=pd, pt0=pt[:, 0:wseg], w0=wseg,
                                   st=first_of_chunk, sp=last_of_chunk,
                                   po0=po, c0=c, t00=t0, nblk0=nblk):
                            nc.tensor.matmul(pd0[0:1, 0:w0], ones_bf, pt0,
                                             start=st, stop=sp)
                            if not sp:
                                return
                            # chunk tail: fold pair halves, then reciprocal
                            rec = dsp.tile([1, TC], F32, tag="rec")
                            if nblk0 > 1:
                                dsum = dsp.tile([1, TC], F32, tag="dsum")
                                nc.vector.tensor_reduce(
                                    out=dsum,
                                    in_=pd0.rearrange("p (k j) -> p j k", k=2),
                                    axis=mybir.AxisListType.X,
                                    op=mybir.AluOpType.add,
                                )
                                nc.vector.reciprocal_approx_fast(out=rec, in_=dsum)
                            else:
                                nc.vector.reciprocal_approx_fast(
                                    out=rec, in_=pd0[0:1, 0:256]
                                )
                            pending_nrm.append((rec, po0, c0, t00))
                            if len(pending_nrm) >= nrm_depth:
                                flush_nrm()
                        if any_masked:
                            pending_diag.append(em_den)
                        else:
                            em_den()

                    flush_diag()
                    while pending_nrm:
                        flush_nrm()
    nc.finalize()
    return nc


def _to_bf16(x):
    import ml_dtypes

    return np.asarray(x, np.float32).astype(ml_dtypes.bfloat16)


def kernel(q, kv, key_padding_mask):
    from concourse.bass_utils import run_bass_kernel_spmd

    q = np.asarray(q, dtype=np.float32)
    kv = np.asarray(kv, dtype=np.float32)
    kpm = np.asarray(key_padding_mask)
    sks = tuple(int(x) for x in kpm.sum(axis=1))

    nc = _build(sks)

    k_all = kv[:, :, 0]  # (B, SK, HKV, D)
    v_all = kv[:, :, 1]
    ident = np.eye(128, dtype=np.float32)

    in_maps = []
    for c in range(N_CORES):
        g, half = c // 2, c % 2
        heads = [4 * g + 2 * half, 4 * g + 2 * half + 1]
        qt = np.ascontiguousarray(
            q[:, :, heads, :].transpose(0, 2, 3, 1)  # (B, 2, D, SQ)
        )
        kt = np.ascontiguousarray(k_all[:, :, g, :].transpose(0, 2, 1))  # (B, D, SK)
        v = np.ascontiguousarray(v_all[:, :, g, :])  # (B, SK, D)
        in_maps.append(
            {
                "qt": _to_bf16(qt),
                "kt": _to_bf16(kt),
                "v": _to_bf16(v),
                "ident": _to_bf16(ident),
            }
        )

    import os

    trace = bool(os.environ.get("BASS_MHA_TRACE"))
    if trace:
        try:
            import trace_hook  # noqa: F401  (dev-only NTFF hook shim)
        except ImportError:
            trace = False

    res = run_bass_kernel_spmd(
        nc, in_maps, list(range(N_CORES)),
        trace=trace, trace_cores=[0] if trace else None,
    )
    kernel._last_exec_time_ns = res.exec_time_ns
    kernel._last_trace = res.instructions_and_trace

    out = np.empty((B, SQ, H, D), dtype=np.float32)
    for c in range(N_CORES):
        g, half = c // 2, c % 2
        heads = [4 * g + 2 * half, 4 * g + 2 * half + 1]
        r = np.asarray(res.results[c]["out"], dtype=np.float32)  # (B, 2, SQ, D)
        for b in range(B):
            for hh, h in enumerate(heads):
                out[b, :, h, :] = r[b, hh]

    # uniform-attention rows: all scores == -10000 -> mean over ALL value rows
    vm = v_all.mean(axis=1)  # (B, HKV, D)
    for b in range(B):
        lo = SQ - sks[b]
        if lo > 0:
            out[b, :lo, :, :] = vm[b, np.arange(H) // (H // HKV), :][None, :, :]
    return out


kernel._last_exec_time_ns = None
kernel._last_trace = None
